# revision 10
# baseline (speedup 1.0000x reference)
"""AttentionHead kernel for 8x TRN2 NeuronCores (Bass/Tile).

Reference semantics (faithful quirk: attention mixes HEADS at each position):
  q = x@Wq.T+bq ; k,v likewise ; reshape [B,S,H,Dk]
  scores[b,s,h,t] = sum_d q[b,s,h,d]*k[b,s,t,d] / sqrt(D)
  attn = softmax_t(scores) ; out[b,s,h,:] = sum_t attn*v[b,s,t,:]
  final = out@Wo.T + bo

Sharding: data-parallel over the 16384 tokens (2048/core).

v3 changes vs v2:
  - DVE load rebalanced: the two AV halving-tree steps (w=8, w=4) and the
    softmax denominator reduce run on the Pool/GPSIMD engine, which was idle;
    DVE keeps the products and the rest.
  - Softmax normalization folded BEFORE the AV product: probs are scaled by
    1/denom (ACT Reciprocal with broadcast) as a 256-elem mul instead of the
    1024-elem post-AV normalize.
  - All large DMAs are 128-row x contiguous-run transfers: the host pre-packs
    x, the weight wall, and unpacks the output so no strided descriptor
    patterns remain on device.
  - The sharded executable is AOT-compiled through fast_dispatch_compile so
    steady-state dispatch takes the C++ fast path.
"""

import hashlib
import numpy as np
import ml_dtypes

import jax
import jax.numpy as jnp
from jax.sharding import Mesh, NamedSharding, PartitionSpec
from jax.experimental.shard_map import shard_map

import concourse.bass as bass
import concourse.mybir as mybir
from concourse import bacc
from concourse.tile import TileContext
from concourse.bass2jax import (
    _bass_exec_p,
    install_neuronx_cc_hook,
    partition_id_tensor,
    fast_dispatch_compile,
)
from concourse.masks import make_identity

BF16 = ml_dtypes.bfloat16

B, S, D = 4, 4096, 1024
H, DK = 16, 64
NCORES = 8
T = B * S                 # 16384 tokens
NCHUNK = 1                # pipeline chunks per call
TPC = T // NCORES // NCHUNK   # 2048 tokens per core
PT = 128                  # tokens per tile (partition dim)
NT = TPC // PT            # 16 tiles per core
LG = 1                    # tiles per x-load group
SG = 2                    # tiles per store group
NGRP = NT // LG           # x load groups

_CACHE = {}


def _build_nc(wall_sb, bias4):
    """wall_sb: [PT, 4*8*D] bf16 host-prepacked in SBUF layout
    (wall_sb[p, ((m*8)+c)*D+o] = W?.T[c*128+p, o] for matrix m), bias4:
    [1, 4D] bf16. Both baked into the NEFF as Consts."""
    nc = bacc.Bacc()
    dt = mybir.dt

    xT = nc.declare_dram_parameter("xT", [PT, NT * 8 * PT], dt.bfloat16,
                                   isOutput=False)
    out = nc.declare_dram_parameter("out", [PT, NT * D], dt.bfloat16,
                                    isOutput=True)
    wall_t = nc.inline_tensor(wall_sb, name="wall")
    bias_t = nc.inline_tensor(bias4, name="bias4")

    inv_sqrt_d = 1.0 / np.sqrt(np.float32(D))

    with TileContext(nc) as tc:
        with (
            tc.tile_pool(name="wpool", bufs=1) as wpool,
            tc.tile_pool(name="xpool", bufs=2) as xpool,
            tc.tile_pool(name="qkv", bufs=2) as qkvpool,
            tc.tile_pool(name="prodp", bufs=1) as prodpool,
            tc.tile_pool(name="prodv2", bufs=2) as prodv2pool,
            tc.tile_pool(name="small", bufs=2) as smallpool,
            tc.tile_pool(name="big", bufs=2) as bigpool,
            tc.tile_pool(name="ppsum", bufs=3, space="PSUM") as ppsum,
            tc.tile_pool(name="tpsum", bufs=2, space="PSUM") as tpsum,
        ):
            # ---- one-time loads (each DMA: 128 rows x 16KB contiguous) ----
            w_sb = wpool.tile([PT, 4, 8, D], dt.bfloat16)
            for m in range(4):
                nc.sync.dma_start(
                    out=w_sb[:, m, :, :].rearrange("p c o -> p (c o)"),
                    in_=wall_t[:, m * 8 * D : (m + 1) * 8 * D],
                )
            bias_sb = wpool.tile([1, 4 * D], dt.bfloat16)
            nc.sync.dma_start(out=bias_sb, in_=bias_t[:, :])
            ones_sb = wpool.tile([1, PT], dt.bfloat16)
            nc.vector.memset(ones_sb, 1.0)
            ident = wpool.tile([PT, PT], dt.bfloat16)
            make_identity(nc, ident)
            zbias = wpool.tile([PT, 1], dt.float32)
            nc.vector.memset(zbias, 0.0)

            xt2 = None
            ctxA = {}     # it -> (v_sb, probs)   consumed by stage1 at it+1
            ctxB = {}     # it -> pv2             consumed by stage2 at it+2
            fout = None
            # two-stage software pipeline over tiles:
            #   front(it):  x load, q/k/v projections, scores, exp
            #   stage1(it-1): softmax-normalize probs, AV product (DVE),
            #                 then the w=8/w=4 tree steps on Pool
            #   stage2(it-2): w=2 + final add (DVE), transpose, O-proj, store
            # stage1 is issued FIRST so Pool gets its work early; the DVE ops
            # that depend on Pool results live a full tile behind, so the
            # in-order DVE queue never stalls on Pool.
            for it in range(NT + 2):
                if 1 <= it <= NT:
                    j1 = it - 1
                    v_sb, probs = ctxA.pop(j1)
                    # ---- softmax denominator + normalize probs ----
                    denom = smallpool.tile([PT, H], dt.float32, tag="denom")
                    nc.vector.tensor_reduce(
                        denom, probs, axis=mybir.AxisListType.X,
                        op=mybir.AluOpType.add,
                    )
                    rden = smallpool.tile([PT, H], dt.float32, tag="rden")
                    nc.vector.reciprocal(rden, denom)
                    probs2 = smallpool.tile([PT, H, H], dt.bfloat16, tag="probs2")
                    rb = bass.AP(
                        tensor=rden.tensor, offset=rden.offset,
                        ap=[rden.ap[0], [1, H], [0, H]],
                    )
                    nc.vector.tensor_mul(probs2, probs, rb)
                    pv2 = prodv2pool.tile([PT, H, DK, 8], dt.bfloat16, tag="pv2")

                    # ---- prodv[s,h,d,t] = probs2[s,h,t] * v[s,t,d] ----
                    # two h-halves; Pool's tree steps chase each half so Pool
                    # starts ~4us into the tile instead of ~12us
                    HH = H // 2

                    def av_half(hh, probs2=probs2, v_sb=v_sb, pv2=pv2):
                        pvh = prodpool.tile([PT, HH, DK, H], dt.bfloat16,
                                            tag=f"prodv{hh}")
                        pb = bass.AP(
                            tensor=probs2.tensor,
                            offset=probs2.offset + hh * HH * H,
                            ap=[probs2.ap[0], [H, HH], [0, DK], [1, H]],
                        )  # [p, 8h, 64d(bcast), 16t]
                        vb = bass.AP(
                            tensor=v_sb.tensor, offset=v_sb.offset,
                            ap=[v_sb.ap[0], [0, HH], [H, DK], [1, H]],
                        )  # [p, 8h(bcast), 64d, 16t]  (v stored d-major)
                        nc.vector.tensor_mul(pvh, pb, vb)
                        p2h = pv2[:, hh * HH : (hh + 1) * HH, :, :]
                        nc.gpsimd.tensor_add(
                            p2h, pvh[:, :, :, 0:8], pvh[:, :, :, 8:16],
                        )
                        nc.gpsimd.tensor_add(
                            p2h[:, :, :, 0:4], p2h[:, :, :, 0:4],
                            p2h[:, :, :, 4:8],
                        )

                    av_half(0)

                # ---- stage2 DVE part for it-2: finish AV reduction ----
                # (sits between the two J halves so it's ready before the
                # front stage and never stalls on Pool)
                if it >= 2:
                    j2 = it - 2
                    pv2b = ctxB.pop(j2)
                    nc.vector.tensor_add(
                        pv2b[:, :, :, 0:2], pv2b[:, :, :, 0:2],
                        pv2b[:, :, :, 2:4],
                    )
                    ao_bf = bigpool.tile([PT, D], dt.bfloat16, tag="aobf")
                    ao3 = ao_bf.rearrange("p (h d) -> p h d", h=H)
                    v0 = bass.AP(
                        tensor=pv2b.tensor, offset=pv2b.offset,
                        ap=[pv2b.ap[0], [DK * 8, H], [8, DK]],
                    )
                    v1 = bass.AP(
                        tensor=pv2b.tensor, offset=pv2b.offset + 1,
                        ap=[pv2b.ap[0], [DK * 8, H], [8, DK]],
                    )
                    nc.vector.tensor_add(ao3, v0, v1)

                if 1 <= it <= NT:
                    av_half(1)
                    ctxB[j1] = pv2

                if it < NT:
                    if it % LG == 0:
                        g = it // LG
                        gw = 8 * LG * PT
                        xt2 = xpool.tile([PT, LG, 8, PT], dt.bfloat16, tag="xt")
                        nc.sync.dma_start(
                            out=xt2.rearrange("p l c s -> p (l c s)"),
                            in_=xT[:, g * gw : (g + 1) * gw],
                        )
                    xt = xt2[:, it % LG, :, :]

                    # ---- projections q,k,v ----
                    qkv_sb = []
                    for m in range(3):
                        dst = qkvpool.tile([PT, D], dt.bfloat16, tag=f"qkv{m}")
                        ps = ppsum.tile([PT, D], dt.float32, tag="ppsum")
                        for half in range(2):
                            off = half * 512
                            psh = ps[:, off : off + 512]
                            nc.tensor.matmul(
                                psh,
                                ones_sb,
                                bias_sb[:, m * D + off : m * D + off + 512],
                                start=True,
                                stop=False,
                            )
                            for c in range(8):
                                nc.tensor.matmul(
                                    psh,
                                    xt[:, c, :],
                                    w_sb[:, m, c, off : off + 512],
                                    start=False,
                                    stop=(c == 7),
                                )
                        nc.scalar.activation(
                            dst, ps, func=mybir.ActivationFunctionType.Copy,
                        )
                        qkv_sb.append(dst)
                    q_sb, k_sb, v_sb = qkv_sb

                    # ---- scores[s,h,t] = sum_d q[s,h,d]*k[s,t,d] ----
                    prod = prodpool.tile([PT, H, H, DK], dt.bfloat16, tag="prod")
                    qb = bass.AP(
                        tensor=q_sb.tensor,
                        offset=q_sb.offset,
                        ap=[q_sb.ap[0], [DK, H], [0, H], [1, DK]],
                    )  # [p, 16h, 16t(bcast), 64d]
                    kb = bass.AP(
                        tensor=k_sb.tensor,
                        offset=k_sb.offset,
                        ap=[k_sb.ap[0], [0, H], [DK, H], [1, DK]],
                    )  # [p, 16h(bcast), 16t, 64d]
                    nc.vector.tensor_mul(prod, qb, kb)
                    w_ = DK
                    while w_ > 2:
                        w_ //= 2
                        nc.vector.tensor_add(
                            prod[:, :, :, 0:w_], prod[:, :, :, 0:w_],
                            prod[:, :, :, w_ : 2 * w_],
                        )
                    scores = smallpool.tile([PT, H, H], dt.float32, tag="scores")
                    p0 = bass.AP(
                        tensor=prod.tensor, offset=prod.offset,
                        ap=[prod.ap[0], [H * DK, H], [DK, H]],
                    )
                    p1 = bass.AP(
                        tensor=prod.tensor, offset=prod.offset + 1,
                        ap=[prod.ap[0], [H * DK, H], [DK, H]],
                    )
                    nc.vector.tensor_add(scores, p0, p1)

                    # ---- exp (softmax numerator; 1/sqrt(D) in the scale) ----
                    probs = smallpool.tile([PT, H, H], dt.bfloat16, tag="probs")
                    nc.scalar.activation(
                        probs, scores, func=mybir.ActivationFunctionType.Exp,
                        bias=zbias[:, 0:1], scale=float(inv_sqrt_d),
                    )
                    ctxA[it] = (v_sb, probs)

                # ---- stage2 PE/ACT part for it-2: transpose + O-proj ----
                if it >= 2:
                    sub = j2 % SG
                    if sub == 0:
                        fout = bigpool.tile([PT, SG, D], dt.bfloat16, tag="fout")
                        # absorbs the WAR-vs-store wait into DVE's clock
                        nc.vector.memset(fout[:, 0, 0:1], 0.0)

                    aoT = bigpool.tile([PT, 8, PT], dt.bfloat16, tag="aoT")
                    tp = tpsum.tile([PT, D], dt.bfloat16, tag="tpsum")
                    for c in range(8):
                        nc.tensor.transpose(
                            tp[:, c * PT : (c + 1) * PT],
                            ao_bf[:, c * PT : (c + 1) * PT],
                            ident,
                        )
                    nc.scalar.activation(
                        aoT.rearrange("p c s -> p (c s)"),
                        tp, func=mybir.ActivationFunctionType.Copy,
                    )
                    ps = ppsum.tile([PT, D], dt.float32, tag="ppsum")
                    for half in range(2):
                        off = half * 512
                        psh = ps[:, off : off + 512]
                        nc.tensor.matmul(
                            psh,
                            ones_sb,
                            bias_sb[:, 3 * D + off : 3 * D + off + 512],
                            start=True,
                            stop=False,
                        )
                        for c in range(8):
                            nc.tensor.matmul(
                                psh,
                                aoT[:, c, :],
                                w_sb[:, 3, c, off : off + 512],
                                start=False,
                                stop=(c == 7),
                            )
                    nc.scalar.activation(
                        fout[:, sub, :], ps,
                        func=mybir.ActivationFunctionType.Copy,
                    )
                    if sub == SG - 1:
                        gt = j2 - sub  # first tile of the store group
                        nc.sync.dma_start(
                            out=out[:, gt * D : (gt + SG) * D],
                            in_=fout.rearrange("p j o -> p (j o)"),
                        )

    nc.compile()
    return nc


def _host_prep_w(Wq, bq, Wk, bk, Wv, bv, Wo, bo):
    perm = (np.arange(D).reshape(H, DK).T).reshape(-1)  # perm[d*16+t] = t*64+d
    wall = np.concatenate(
        [Wq.T, Wk.T, Wv.T[:, perm], Wo.T], axis=1
    ).astype(BF16)  # [D, 4D]
    # repack to SBUF layout: wall_sb[p, (m c o)] = wall[c*128+p, m*1024+o]
    wall_sb = np.ascontiguousarray(
        wall.reshape(8, PT, 4, D).transpose(1, 2, 0, 3).reshape(PT, 4 * 8 * D)
    )
    bias4 = np.concatenate([bq, bk, bv[perm], bo]).astype(BF16)[None, :]
    return wall_sb, np.ascontiguousarray(bias4)


def host_pack_x(x_core):
    """x_core: [TPC, D] fp32/bf16 for ONE core -> [PT, NT*8*PT] bf16 in the
    on-device xT layout: xT[p, ((g*LG+l)*8+c)*PT+s] = x[(g*LG+l)*PT+s, c*PT+p]
    i.e. grouped by LG-tile load group, channel-chunk major, token minor."""
    xg = x_core.reshape(NGRP, LG * PT, 8, PT)        # [g, s_local, c, p]
    xg = xg.transpose(3, 0, 2, 1)                    # [p, g, c, s_local]
    # within a group the SBUF tile is [p, l, c, s]: split s_local = (l, s)
    xg = xg.reshape(PT, NGRP, 8, LG, PT).transpose(0, 1, 3, 2, 4)
    return np.ascontiguousarray(xg.reshape(PT, NT * 8 * PT)).astype(BF16)


def host_unpack_out(res_core):
    """res_core: [PT, NT*D] bf16 -> [TPC, D] fp32."""
    o = res_core.reshape(PT, NT, D).transpose(1, 0, 2)
    return o.reshape(TPC, D).astype(np.float32)


def _make_runner(nc):
    """fast-dispatch jit-of-shard_map runner over 8 cores with
    device-created, donated output buffers."""
    install_neuronx_cc_hook()

    partition_name = (
        nc.partition_id_tensor.name if nc.partition_id_tensor else None
    )
    in_names, out_names, out_avals = [], [], []
    for alloc in nc.m.functions[0].allocations:
        if not isinstance(alloc, mybir.MemoryLocationSet):
            continue
        name = alloc.memorylocations[0].name
        if alloc.kind == "ExternalInput":
            if name != partition_name:
                in_names.append(name)
        elif alloc.kind == "ExternalOutput":
            out_names.append(name)
            shape = tuple(alloc.tensor_shape)
            dtype = mybir.dt.np(alloc.dtype)
            out_avals.append(jax.core.ShapedArray(shape, dtype))
    assert in_names == ["xT"] and out_names == ["out"], (in_names, out_names)

    all_in_names = in_names + out_names
    if partition_name is not None:
        all_in_names.append(partition_name)

    def _body(*args):
        operands = list(args)
        if partition_name is not None:
            operands.append(partition_id_tensor())
        outs = _bass_exec_p.bind(
            *operands,
            out_avals=tuple(out_avals),
            in_names=tuple(all_in_names),
            out_names=tuple(out_names),
            lowering_input_output_aliases=(),
            sim_require_finite=True,
            sim_require_nnan=True,
            nc=nc,
        )
        return tuple(outs)

    devices = jax.devices()[:NCORES]
    mesh = Mesh(np.asarray(devices), ("core",))
    sh = NamedSharding(mesh, PartitionSpec("core"))
    ishape = (NCORES * PT, NT * 8 * PT)
    oshape = (NCORES * PT, NT * D)
    odtype = out_avals[0].dtype

    def _compile():
        return (
            jax.jit(
                shard_map(
                    _body, mesh=mesh,
                    in_specs=(PartitionSpec("core"),) * 2,
                    out_specs=(PartitionSpec("core"),),
                    check_rep=False,
                ),
                donate_argnums=(1,),
                keep_unused=True,
            )
            .lower(
                jax.ShapeDtypeStruct(ishape, BF16, sharding=sh),
                jax.ShapeDtypeStruct(oshape, odtype, sharding=sh),
            )
            .compile()
        )

    try:
        sharded = fast_dispatch_compile(_compile)
    except Exception:
        sharded = jax.jit(
            shard_map(
                _body, mesh=mesh,
                in_specs=(PartitionSpec("core"),) * 2,
                out_specs=(PartitionSpec("core"),),
                check_rep=False,
            ),
            donate_argnums=(1,),
            keep_unused=True,
        )

    zeros_maker = jax.jit(
        lambda: jnp.zeros(oshape, odtype), out_shardings=sh,
    )

    obuf_cell = {"o": []}

    def run(x_chunks):
        """x_chunks: list of NCHUNK np arrays [NCORES*PT, NT*8*PT] bf16."""
        obufs = obuf_cell["o"]
        while len(obufs) < len(x_chunks):
            obufs.append(zeros_maker())
        outs = []
        for j, xc in enumerate(x_chunks):
            xd = jax.device_put(xc, sh)
            try:
                (o,) = sharded(xd, obufs[j])
            except Exception:
                (o,) = sharded(xd, zeros_maker())
            try:
                o.copy_to_host_async()
            except Exception:
                pass
            outs.append(o)
        res = [np.asarray(o) for o in outs]
        obuf_cell["o"] = outs  # donated next call (overwritten fully)
        return res

    return run


def kernel(x, Wq, bq, Wk, bk, Wv, bv, Wo, bo):
    x = np.asarray(x, dtype=np.float32)
    arrs = [np.asarray(a, dtype=np.float32)
            for a in (Wq, bq, Wk, bk, Wv, bv, Wo, bo)]

    wkey = hashlib.sha256(b"".join(a.tobytes() for a in arrs)).hexdigest()
    if _CACHE.get("wkey") != wkey:
        wall_sb, bias4 = _host_prep_w(*arrs)
        nc = _build_nc(wall_sb, bias4)
        _CACHE.update(
            wkey=wkey, nc=nc, run=_make_runner(nc),
        )

    x4 = x.reshape(NCORES, NCHUNK, TPC, D)
    x_chunks = [
        np.concatenate(
            [host_pack_x(x4[core, j]) for core in range(NCORES)], axis=0
        )
        for j in range(NCHUNK)
    ]
    res = _CACHE["run"](x_chunks)  # NCHUNK x [NCORES*PT, NT*D] bf16
    out = np.empty((NCORES, NCHUNK, TPC, D), np.float32)
    for j in range(NCHUNK):
        r = res[j].reshape(NCORES, PT, NT * D)
        for core in range(NCORES):
            out[core, j] = host_unpack_out(r[core])
    return out.reshape(B, S, D)


# revision 11
# speedup vs baseline: 1.1287x; 1.1287x over previous
"""AttentionHead kernel for 8x TRN2 NeuronCores (Bass/Tile).

Reference semantics (faithful quirk: attention mixes HEADS at each position):
  q = x@Wq.T+bq ; k,v likewise ; reshape [B,S,H,Dk]
  scores[b,s,h,t] = sum_d q[b,s,h,d]*k[b,s,t,d] / sqrt(D)
  attn = softmax_t(scores) ; out[b,s,h,:] = sum_t attn*v[b,s,t,:]
  final = out@Wo.T + bo

Sharding: data-parallel over the 16384 tokens (2048/core).

v3 changes vs v2:
  - DVE load rebalanced: the two AV halving-tree steps (w=8, w=4) and the
    softmax denominator reduce run on the Pool/GPSIMD engine, which was idle;
    DVE keeps the products and the rest.
  - Softmax normalization folded BEFORE the AV product: probs are scaled by
    1/denom (ACT Reciprocal with broadcast) as a 256-elem mul instead of the
    1024-elem post-AV normalize.
  - All large DMAs are 128-row x contiguous-run transfers: the host pre-packs
    x, the weight wall, and unpacks the output so no strided descriptor
    patterns remain on device.
  - The sharded executable is AOT-compiled through fast_dispatch_compile so
    steady-state dispatch takes the C++ fast path.
"""

import hashlib
import numpy as np
import ml_dtypes

import jax
import jax.numpy as jnp
from jax.sharding import Mesh, NamedSharding, PartitionSpec
from jax.experimental.shard_map import shard_map

import concourse.bass as bass
import concourse.mybir as mybir
from concourse import bacc
from concourse.tile import TileContext
from concourse.bass2jax import (
    _bass_exec_p,
    install_neuronx_cc_hook,
    partition_id_tensor,
    fast_dispatch_compile,
)
from concourse.masks import make_identity

BF16 = ml_dtypes.bfloat16

B, S, D = 4, 4096, 1024
H, DK = 16, 64
NCORES = 8
T = B * S                 # 16384 tokens
NCHUNK = 1                # pipeline chunks per call
TPC = T // NCORES // NCHUNK   # 2048 tokens per core
PT = 128                  # tokens per tile (partition dim)
NT = TPC // PT            # 16 tiles per core
LG = 1                    # tiles per x-load group
SG = 2                    # tiles per store group
NGRP = NT // LG           # x load groups

_CACHE = {}


def _build_nc(wall_sb, bias4):
    """wall_sb: [PT, 4*8*D] bf16 host-prepacked in SBUF layout
    (wall_sb[p, ((m*8)+c)*D+o] = W?.T[c*128+p, o] for matrix m), bias4:
    [1, 4D] bf16. Both baked into the NEFF as Consts."""
    nc = bacc.Bacc()
    dt = mybir.dt

    xT = nc.declare_dram_parameter("xT", [PT, NT * 8 * PT], dt.bfloat16,
                                   isOutput=False)
    out = nc.declare_dram_parameter("out", [PT, NT * D], dt.bfloat16,
                                    isOutput=True)
    wall_t = nc.inline_tensor(wall_sb, name="wall")
    bias_t = nc.inline_tensor(bias4, name="bias4")

    inv_sqrt_d = 1.0 / np.sqrt(np.float32(D))

    with TileContext(nc) as tc:
        with (
            tc.tile_pool(name="wpool", bufs=1) as wpool,
            tc.tile_pool(name="xpool", bufs=2) as xpool,
            tc.tile_pool(name="qkv", bufs=2) as qkvpool,
            tc.tile_pool(name="prodp", bufs=1) as prodpool,
            tc.tile_pool(name="prodv2", bufs=2) as prodv2pool,
            tc.tile_pool(name="small", bufs=2) as smallpool,
            tc.tile_pool(name="big", bufs=2) as bigpool,
            tc.tile_pool(name="ppsum", bufs=3, space="PSUM") as ppsum,
            tc.tile_pool(name="tpsum", bufs=2, space="PSUM") as tpsum,
        ):
            # ---- one-time loads (each DMA: 128 rows x 16KB contiguous) ----
            w_sb = wpool.tile([PT, 4, 8, D], dt.bfloat16)
            for m in range(4):
                nc.sync.dma_start(
                    out=w_sb[:, m, :, :].rearrange("p c o -> p (c o)"),
                    in_=wall_t[:, m * 8 * D : (m + 1) * 8 * D],
                )
            bias_sb = wpool.tile([1, 4 * D], dt.bfloat16)
            nc.sync.dma_start(out=bias_sb, in_=bias_t[:, :])
            ones_sb = wpool.tile([1, PT], dt.bfloat16)
            nc.vector.memset(ones_sb, 1.0)
            ident = wpool.tile([PT, PT], dt.bfloat16)
            make_identity(nc, ident)
            zbias = wpool.tile([PT, 1], dt.float32)
            nc.vector.memset(zbias, 0.0)

            xt2 = None
            ctxA = {}     # it -> (v_sb, probs)   consumed by stage1 at it+1
            ctxB = {}     # it -> pv2             consumed by stage2 at it+2
            fout = None
            # two-stage software pipeline over tiles:
            #   front(it):  x load, q/k/v projections, scores, exp
            #   stage1(it-1): softmax-normalize probs, AV product (DVE),
            #                 then the w=8/w=4 tree steps on Pool
            #   stage2(it-2): w=2 + final add (DVE), transpose, O-proj, store
            # stage1 is issued FIRST so Pool gets its work early; the DVE ops
            # that depend on Pool results live a full tile behind, so the
            # in-order DVE queue never stalls on Pool.
            for it in range(NT + 2):
                if 1 <= it <= NT:
                    j1 = it - 1
                    v_sb, probs = ctxA.pop(j1)
                    # ---- softmax denominator + normalize probs ----
                    denom = smallpool.tile([PT, H], dt.float32, tag="denom")
                    nc.vector.tensor_reduce(
                        denom, probs, axis=mybir.AxisListType.X,
                        op=mybir.AluOpType.add,
                    )
                    rden = smallpool.tile([PT, H], dt.float32, tag="rden")
                    nc.vector.reciprocal(rden, denom)
                    probs2 = smallpool.tile([PT, H, H], dt.bfloat16, tag="probs2")
                    rb = bass.AP(
                        tensor=rden.tensor, offset=rden.offset,
                        ap=[rden.ap[0], [1, H], [0, H]],
                    )
                    nc.vector.tensor_mul(probs2, probs, rb)
                    pv2 = prodv2pool.tile([PT, H, DK, 8], dt.bfloat16, tag="pv2")

                    # ---- prodv[s,h,d,t] = probs2[s,h,t] * v[s,t,d] ----
                    # two h-halves; Pool's tree steps chase each half so Pool
                    # starts ~4us into the tile instead of ~12us
                    HH = H // 2

                    def av_half(hh, probs2=probs2, v_sb=v_sb, pv2=pv2):
                        pvh = prodpool.tile([PT, HH, DK, H], dt.bfloat16,
                                            tag=f"prodv{hh}")
                        pb = bass.AP(
                            tensor=probs2.tensor,
                            offset=probs2.offset + hh * HH * H,
                            ap=[probs2.ap[0], [H, HH], [0, DK], [1, H]],
                        )  # [p, 8h, 64d(bcast), 16t]
                        vb = bass.AP(
                            tensor=v_sb.tensor, offset=v_sb.offset,
                            ap=[v_sb.ap[0], [0, HH], [H, DK], [1, H]],
                        )  # [p, 8h(bcast), 64d, 16t]  (v stored d-major)
                        nc.vector.tensor_mul(pvh, pb, vb)
                        p2h = pv2[:, hh * HH : (hh + 1) * HH, :, :]
                        nc.gpsimd.tensor_add(
                            p2h, pvh[:, :, :, 0:8], pvh[:, :, :, 8:16],
                        )
                        nc.gpsimd.tensor_add(
                            p2h[:, :, :, 0:4], p2h[:, :, :, 0:4],
                            p2h[:, :, :, 4:8],
                        )

                    av_half(0)

                if 1 <= it <= NT:
                    av_half(1)
                    ctxB[j1] = pv2

                if it < NT:
                    if it % LG == 0:
                        g = it // LG
                        gw = 8 * LG * PT
                        xt2 = xpool.tile([PT, LG, 8, PT], dt.bfloat16, tag="xt")
                        nc.sync.dma_start(
                            out=xt2.rearrange("p l c s -> p (l c s)"),
                            in_=xT[:, g * gw : (g + 1) * gw],
                        )
                    xt = xt2[:, it % LG, :, :]

                    # ---- projections q,k,v ----
                    qkv_sb = []
                    for m in range(3):
                        dst = qkvpool.tile([PT, D], dt.bfloat16, tag=f"qkv{m}")
                        ps = ppsum.tile([PT, D], dt.float32, tag="ppsum")
                        for half in range(2):
                            off = half * 512
                            psh = ps[:, off : off + 512]
                            nc.tensor.matmul(
                                psh,
                                ones_sb,
                                bias_sb[:, m * D + off : m * D + off + 512],
                                start=True,
                                stop=False,
                            )
                            for c in range(8):
                                nc.tensor.matmul(
                                    psh,
                                    xt[:, c, :],
                                    w_sb[:, m, c, off : off + 512],
                                    start=False,
                                    stop=(c == 7),
                                )
                        nc.scalar.activation(
                            dst, ps, func=mybir.ActivationFunctionType.Copy,
                        )
                        qkv_sb.append(dst)
                    q_sb, k_sb, v_sb = qkv_sb

                    # ---- scores[s,h,t] = sum_d q[s,h,d]*k[s,t,d] ----
                    prod = prodpool.tile([PT, H, H, DK], dt.bfloat16, tag="prod")
                    qb = bass.AP(
                        tensor=q_sb.tensor,
                        offset=q_sb.offset,
                        ap=[q_sb.ap[0], [DK, H], [0, H], [1, DK]],
                    )  # [p, 16h, 16t(bcast), 64d]
                    kb = bass.AP(
                        tensor=k_sb.tensor,
                        offset=k_sb.offset,
                        ap=[k_sb.ap[0], [0, H], [DK, H], [1, DK]],
                    )  # [p, 16h(bcast), 16t, 64d]
                    nc.vector.tensor_mul(prod, qb, kb)
                    w_ = DK
                    while w_ > 2:
                        w_ //= 2
                        nc.vector.tensor_add(
                            prod[:, :, :, 0:w_], prod[:, :, :, 0:w_],
                            prod[:, :, :, w_ : 2 * w_],
                        )
                    scores = smallpool.tile([PT, H, H], dt.float32, tag="scores")
                    p0 = bass.AP(
                        tensor=prod.tensor, offset=prod.offset,
                        ap=[prod.ap[0], [H * DK, H], [DK, H]],
                    )
                    p1 = bass.AP(
                        tensor=prod.tensor, offset=prod.offset + 1,
                        ap=[prod.ap[0], [H * DK, H], [DK, H]],
                    )
                    nc.vector.tensor_add(scores, p0, p1)

                    # ---- exp (softmax numerator; 1/sqrt(D) in the scale) ----
                    probs = smallpool.tile([PT, H, H], dt.bfloat16, tag="probs")
                    nc.scalar.activation(
                        probs, scores, func=mybir.ActivationFunctionType.Exp,
                        bias=zbias[:, 0:1], scale=float(inv_sqrt_d),
                    )
                    ctxA[it] = (v_sb, probs)

                # ---- stage2 for it-2: finish AV reduction, transpose,
                # O-projection, store ----
                if it >= 2:
                    j2 = it - 2
                    pv2b = ctxB.pop(j2)
                    nc.vector.tensor_add(
                        pv2b[:, :, :, 0:2], pv2b[:, :, :, 0:2],
                        pv2b[:, :, :, 2:4],
                    )
                    ao_bf = bigpool.tile([PT, D], dt.bfloat16, tag="aobf")
                    ao3 = ao_bf.rearrange("p (h d) -> p h d", h=H)
                    v0 = bass.AP(
                        tensor=pv2b.tensor, offset=pv2b.offset,
                        ap=[pv2b.ap[0], [DK * 8, H], [8, DK]],
                    )
                    v1 = bass.AP(
                        tensor=pv2b.tensor, offset=pv2b.offset + 1,
                        ap=[pv2b.ap[0], [DK * 8, H], [8, DK]],
                    )
                    nc.vector.tensor_add(ao3, v0, v1)
                    sub = j2 % SG
                    if sub == 0:
                        fout = bigpool.tile([PT, SG, D], dt.bfloat16, tag="fout")
                        # absorbs the WAR-vs-store wait into DVE's clock
                        nc.vector.memset(fout[:, 0, 0:1], 0.0)

                    aoT = bigpool.tile([PT, 8, PT], dt.bfloat16, tag="aoT")
                    tp = tpsum.tile([PT, D], dt.bfloat16, tag="tpsum")
                    for c in range(8):
                        nc.tensor.transpose(
                            tp[:, c * PT : (c + 1) * PT],
                            ao_bf[:, c * PT : (c + 1) * PT],
                            ident,
                        )
                    nc.scalar.activation(
                        aoT.rearrange("p c s -> p (c s)"),
                        tp, func=mybir.ActivationFunctionType.Copy,
                    )
                    ps = ppsum.tile([PT, D], dt.float32, tag="ppsum")
                    for half in range(2):
                        off = half * 512
                        psh = ps[:, off : off + 512]
                        nc.tensor.matmul(
                            psh,
                            ones_sb,
                            bias_sb[:, 3 * D + off : 3 * D + off + 512],
                            start=True,
                            stop=False,
                        )
                        for c in range(8):
                            nc.tensor.matmul(
                                psh,
                                aoT[:, c, :],
                                w_sb[:, 3, c, off : off + 512],
                                start=False,
                                stop=(c == 7),
                            )
                    nc.scalar.activation(
                        fout[:, sub, :], ps,
                        func=mybir.ActivationFunctionType.Copy,
                    )
                    if sub == SG - 1:
                        gt = j2 - sub  # first tile of the store group
                        nc.sync.dma_start(
                            out=out[:, gt * D : (gt + SG) * D],
                            in_=fout.rearrange("p j o -> p (j o)"),
                        )

    nc.compile()
    return nc


def _host_prep_w(Wq, bq, Wk, bk, Wv, bv, Wo, bo):
    perm = (np.arange(D).reshape(H, DK).T).reshape(-1)  # perm[d*16+t] = t*64+d
    wall = np.concatenate(
        [Wq.T, Wk.T, Wv.T[:, perm], Wo.T], axis=1
    ).astype(BF16)  # [D, 4D]
    # repack to SBUF layout: wall_sb[p, (m c o)] = wall[c*128+p, m*1024+o]
    wall_sb = np.ascontiguousarray(
        wall.reshape(8, PT, 4, D).transpose(1, 2, 0, 3).reshape(PT, 4 * 8 * D)
    )
    bias4 = np.concatenate([bq, bk, bv[perm], bo]).astype(BF16)[None, :]
    return wall_sb, np.ascontiguousarray(bias4)


def host_pack_x(x_core):
    """x_core: [TPC, D] fp32/bf16 for ONE core -> [PT, NT*8*PT] bf16 in the
    on-device xT layout: xT[p, ((g*LG+l)*8+c)*PT+s] = x[(g*LG+l)*PT+s, c*PT+p]
    i.e. grouped by LG-tile load group, channel-chunk major, token minor."""
    xg = x_core.reshape(NGRP, LG * PT, 8, PT)        # [g, s_local, c, p]
    xg = xg.transpose(3, 0, 2, 1)                    # [p, g, c, s_local]
    # within a group the SBUF tile is [p, l, c, s]: split s_local = (l, s)
    xg = xg.reshape(PT, NGRP, 8, LG, PT).transpose(0, 1, 3, 2, 4)
    return np.ascontiguousarray(xg.reshape(PT, NT * 8 * PT)).astype(BF16)


def host_unpack_out(res_core):
    """res_core: [PT, NT*D] bf16 -> [TPC, D] fp32."""
    o = res_core.reshape(PT, NT, D).transpose(1, 0, 2)
    return o.reshape(TPC, D).astype(np.float32)


def _make_runner(nc):
    """fast-dispatch jit-of-shard_map runner over 8 cores with
    device-created, donated output buffers."""
    install_neuronx_cc_hook()

    partition_name = (
        nc.partition_id_tensor.name if nc.partition_id_tensor else None
    )
    in_names, out_names, out_avals = [], [], []
    for alloc in nc.m.functions[0].allocations:
        if not isinstance(alloc, mybir.MemoryLocationSet):
            continue
        name = alloc.memorylocations[0].name
        if alloc.kind == "ExternalInput":
            if name != partition_name:
                in_names.append(name)
        elif alloc.kind == "ExternalOutput":
            out_names.append(name)
            shape = tuple(alloc.tensor_shape)
            dtype = mybir.dt.np(alloc.dtype)
            out_avals.append(jax.core.ShapedArray(shape, dtype))
    assert in_names == ["xT"] and out_names == ["out"], (in_names, out_names)

    all_in_names = in_names + out_names
    if partition_name is not None:
        all_in_names.append(partition_name)

    def _body(*args):
        operands = list(args)
        if partition_name is not None:
            operands.append(partition_id_tensor())
        outs = _bass_exec_p.bind(
            *operands,
            out_avals=tuple(out_avals),
            in_names=tuple(all_in_names),
            out_names=tuple(out_names),
            lowering_input_output_aliases=(),
            sim_require_finite=True,
            sim_require_nnan=True,
            nc=nc,
        )
        return tuple(outs)

    devices = jax.devices()[:NCORES]
    mesh = Mesh(np.asarray(devices), ("core",))
    sh = NamedSharding(mesh, PartitionSpec("core"))
    ishape = (NCORES * PT, NT * 8 * PT)
    oshape = (NCORES * PT, NT * D)
    odtype = out_avals[0].dtype

    def _compile():
        return (
            jax.jit(
                shard_map(
                    _body, mesh=mesh,
                    in_specs=(PartitionSpec("core"),) * 2,
                    out_specs=(PartitionSpec("core"),),
                    check_rep=False,
                ),
                donate_argnums=(1,),
                keep_unused=True,
            )
            .lower(
                jax.ShapeDtypeStruct(ishape, BF16, sharding=sh),
                jax.ShapeDtypeStruct(oshape, odtype, sharding=sh),
            )
            .compile()
        )

    try:
        sharded = fast_dispatch_compile(_compile)
    except Exception:
        sharded = jax.jit(
            shard_map(
                _body, mesh=mesh,
                in_specs=(PartitionSpec("core"),) * 2,
                out_specs=(PartitionSpec("core"),),
                check_rep=False,
            ),
            donate_argnums=(1,),
            keep_unused=True,
        )

    zeros_maker = jax.jit(
        lambda: jnp.zeros(oshape, odtype), out_shardings=sh,
    )

    obuf_cell = {"o": []}

    def run(x_chunks):
        """x_chunks: list of NCHUNK np arrays [NCORES*PT, NT*8*PT] bf16."""
        obufs = obuf_cell["o"]
        while len(obufs) < len(x_chunks):
            obufs.append(zeros_maker())
        outs = []
        for j, xc in enumerate(x_chunks):
            xd = jax.device_put(xc, sh)
            try:
                (o,) = sharded(xd, obufs[j])
            except Exception:
                (o,) = sharded(xd, zeros_maker())
            try:
                o.copy_to_host_async()
            except Exception:
                pass
            outs.append(o)
        res = [np.asarray(o) for o in outs]
        obuf_cell["o"] = outs  # donated next call (overwritten fully)
        return res

    return run


def kernel(x, Wq, bq, Wk, bk, Wv, bv, Wo, bo):
    x = np.asarray(x, dtype=np.float32)
    arrs = [np.asarray(a, dtype=np.float32)
            for a in (Wq, bq, Wk, bk, Wv, bv, Wo, bo)]

    wkey = hashlib.sha256(b"".join(a.tobytes() for a in arrs)).hexdigest()
    if _CACHE.get("wkey") != wkey:
        wall_sb, bias4 = _host_prep_w(*arrs)
        nc = _build_nc(wall_sb, bias4)
        _CACHE.update(
            wkey=wkey, nc=nc, run=_make_runner(nc),
        )

    x4 = x.reshape(NCORES, NCHUNK, TPC, D)
    x_chunks = [
        np.concatenate(
            [host_pack_x(x4[core, j]) for core in range(NCORES)], axis=0
        )
        for j in range(NCHUNK)
    ]
    res = _CACHE["run"](x_chunks)  # NCHUNK x [NCORES*PT, NT*D] bf16
    out = np.empty((NCORES, NCHUNK, TPC, D), np.float32)
    for j in range(NCHUNK):
        r = res[j].reshape(NCORES, PT, NT * D)
        for core in range(NCORES):
            out[core, j] = host_unpack_out(r[core])
    return out.reshape(B, S, D)


# revision 13
# speedup vs baseline: 1.1534x; 1.0219x over previous
"""AttentionHead kernel for 8x TRN2 NeuronCores (Bass/Tile).

Reference semantics (faithful quirk: attention mixes HEADS at each position):
  q = x@Wq.T+bq ; k,v likewise ; reshape [B,S,H,Dk]
  scores[b,s,h,t] = sum_d q[b,s,h,d]*k[b,s,t,d] / sqrt(D)
  attn = softmax_t(scores) ; out[b,s,h,:] = sum_t attn*v[b,s,t,:]
  final = out@Wo.T + bo

Sharding: data-parallel over the 16384 tokens (2048/core).

v3 changes vs v2 (sim 633us -> 612us per chunk, plus dispatch-path savings):
  - DVE load rebalanced: the AV halving-tree steps w=8 and w=4 run on the
    Pool/GPSIMD engine (was idle; sim-busy now DVE 466us / Pool 397us).
    The AV product is issued in two h-halves, each chased by its Pool tree
    steps, so Pool starts ~4us into each tile instead of ~12us.
  - Three-stage software pipeline (front / stage1 / stage2, one tile apart):
    the DVE ops that consume Pool results run two tiles behind the producer,
    so the in-order DVE queue never stalls on Pool.
  - Softmax normalization folded BEFORE the AV product: probs scaled by
    1/denom (broadcast mul, 256 elems) instead of the 1024-elem post-AV
    normalize.
  - All large DMAs are 128-row x >=2KB-contiguous-run transfers: the host
    pre-packs x and the weight wall into SBUF layout and unpacks the output,
    so no strided descriptor patterns remain on device.
  - The sharded executable is AOT-compiled through fast_dispatch_compile so
    steady-state dispatch takes the C++ fast path (~100us/call vs ~500us).
"""

import hashlib
import numpy as np
import ml_dtypes

import jax
import jax.numpy as jnp
from jax.sharding import Mesh, NamedSharding, PartitionSpec
from jax.experimental.shard_map import shard_map

import concourse.bass as bass
import concourse.mybir as mybir
from concourse import bacc
from concourse.tile import TileContext
from concourse.bass2jax import (
    _bass_exec_p,
    install_neuronx_cc_hook,
    partition_id_tensor,
    fast_dispatch_compile,
)
from concourse.masks import make_identity

BF16 = ml_dtypes.bfloat16

B, S, D = 4, 4096, 1024
H, DK = 16, 64
NCORES = 8
T = B * S                 # 16384 tokens
NCHUNK = 1                # pipeline chunks per call
TPC = T // NCORES // NCHUNK   # 2048 tokens per core
PT = 128                  # tokens per tile (partition dim)
NT = TPC // PT            # 16 tiles per core
LG = 1                    # tiles per x-load group
SG = 2                    # tiles per store group
NGRP = NT // LG           # x load groups

_CACHE = {}


def _build_nc(wall_sb, bias4):
    """wall_sb: [PT, 4*8*D] bf16 host-prepacked in SBUF layout
    (wall_sb[p, ((m*8)+c)*D+o] = W?.T[c*128+p, o] for matrix m), bias4:
    [1, 4D] bf16. Both baked into the NEFF as Consts."""
    nc = bacc.Bacc()
    dt = mybir.dt

    xT = nc.declare_dram_parameter("xT", [PT, NT * 8 * PT], dt.bfloat16,
                                   isOutput=False)
    out = nc.declare_dram_parameter("out", [PT, NT * D], dt.bfloat16,
                                    isOutput=True)
    wall_t = nc.inline_tensor(wall_sb, name="wall")
    bias_t = nc.inline_tensor(bias4, name="bias4")

    inv_sqrt_d = 1.0 / np.sqrt(np.float32(D))

    with TileContext(nc) as tc:
        with (
            tc.tile_pool(name="wpool", bufs=1) as wpool,
            tc.tile_pool(name="xpool", bufs=2) as xpool,
            tc.tile_pool(name="qkv", bufs=2) as qkvpool,
            tc.tile_pool(name="prodp", bufs=1) as prodpool,
            tc.tile_pool(name="prodv2", bufs=2) as prodv2pool,
            tc.tile_pool(name="small", bufs=2) as smallpool,
            tc.tile_pool(name="big", bufs=2) as bigpool,
            tc.tile_pool(name="ppsum", bufs=3, space="PSUM") as ppsum,
            tc.tile_pool(name="tpsum", bufs=2, space="PSUM") as tpsum,
        ):
            # ---- one-time loads (each DMA: 128 rows x 16KB contiguous) ----
            w_sb = wpool.tile([PT, 4, 8, D], dt.bfloat16)
            for m in range(4):
                nc.sync.dma_start(
                    out=w_sb[:, m, :, :].rearrange("p c o -> p (c o)"),
                    in_=wall_t[:, m * 8 * D : (m + 1) * 8 * D],
                )
            bias_sb = wpool.tile([1, 4 * D], dt.bfloat16)
            nc.sync.dma_start(out=bias_sb, in_=bias_t[:, :])
            ones_sb = wpool.tile([1, PT], dt.bfloat16)
            nc.vector.memset(ones_sb, 1.0)
            ident = wpool.tile([PT, PT], dt.bfloat16)
            make_identity(nc, ident)
            zbias = wpool.tile([PT, 1], dt.float32)
            nc.vector.memset(zbias, 0.0)

            xt2 = None
            ctxA = {}     # it -> (v_sb, probs)   consumed by stage1 at it+1
            ctxB = {}     # it -> pv2             consumed by stage2 at it+2
            fout = None
            # three-stage software pipeline over tiles, issued in the order
            #   stage1(it-1): normalize probs, AV product halves (DVE) each
            #                 chased by its w=8/w=4 tree steps on Pool
            #   front(it):    x load, q/k/v projections, scores, exp
            #   stage2(it-2): w=2 + final add (DVE), transpose, O-proj, store
            # stage1 is issued FIRST so Pool gets its work early; stage2's
            # DVE ops consume Pool results from TWO tiles back, so the
            # in-order DVE queue never stalls on Pool.
            for it in range(NT + 2):
                if 1 <= it <= NT:
                    j1 = it - 1
                    v_sb, probs = ctxA.pop(j1)
                    # ---- softmax denominator + normalize probs ----
                    denom = smallpool.tile([PT, H], dt.float32, tag="denom")
                    nc.vector.tensor_reduce(
                        denom, probs, axis=mybir.AxisListType.X,
                        op=mybir.AluOpType.add,
                    )
                    rden = smallpool.tile([PT, H], dt.float32, tag="rden")
                    nc.vector.reciprocal(rden, denom)
                    probs2 = smallpool.tile([PT, H, H], dt.bfloat16, tag="probs2")
                    rb = bass.AP(
                        tensor=rden.tensor, offset=rden.offset,
                        ap=[rden.ap[0], [1, H], [0, H]],
                    )
                    nc.vector.tensor_mul(probs2, probs, rb)
                    pv2 = prodv2pool.tile([PT, H, DK, 8], dt.bfloat16, tag="pv2")

                    # ---- prodv[s,h,d,t] = probs2[s,h,t] * v[s,t,d] ----
                    # two h-halves; Pool's tree steps chase each half so Pool
                    # starts ~4us into the tile instead of ~12us
                    HH = H // 2

                    def av_half(hh, probs2=probs2, v_sb=v_sb, pv2=pv2):
                        pvh = prodpool.tile([PT, HH, DK, H], dt.bfloat16,
                                            tag=f"prodv{hh}")
                        pb = bass.AP(
                            tensor=probs2.tensor,
                            offset=probs2.offset + hh * HH * H,
                            ap=[probs2.ap[0], [H, HH], [0, DK], [1, H]],
                        )  # [p, 8h, 64d(bcast), 16t]
                        vb = bass.AP(
                            tensor=v_sb.tensor, offset=v_sb.offset,
                            ap=[v_sb.ap[0], [0, HH], [H, DK], [1, H]],
                        )  # [p, 8h(bcast), 64d, 16t]  (v stored d-major)
                        nc.vector.tensor_mul(pvh, pb, vb)
                        p2h = pv2[:, hh * HH : (hh + 1) * HH, :, :]
                        nc.gpsimd.tensor_add(
                            p2h, pvh[:, :, :, 0:8], pvh[:, :, :, 8:16],
                        )
                        nc.gpsimd.tensor_add(
                            p2h[:, :, :, 0:4], p2h[:, :, :, 0:4],
                            p2h[:, :, :, 4:8],
                        )

                    av_half(0)

                if 1 <= it <= NT:
                    av_half(1)
                    ctxB[j1] = pv2

                if it < NT:
                    if it % LG == 0:
                        g = it // LG
                        gw = 8 * LG * PT
                        xt2 = xpool.tile([PT, LG, 8, PT], dt.bfloat16, tag="xt")
                        nc.sync.dma_start(
                            out=xt2.rearrange("p l c s -> p (l c s)"),
                            in_=xT[:, g * gw : (g + 1) * gw],
                        )
                    xt = xt2[:, it % LG, :, :]

                    # ---- projections q,k,v ----
                    qkv_sb = []
                    for m in range(3):
                        dst = qkvpool.tile([PT, D], dt.bfloat16, tag=f"qkv{m}")
                        ps = ppsum.tile([PT, D], dt.float32, tag="ppsum")
                        for half in range(2):
                            off = half * 512
                            psh = ps[:, off : off + 512]
                            nc.tensor.matmul(
                                psh,
                                ones_sb,
                                bias_sb[:, m * D + off : m * D + off + 512],
                                start=True,
                                stop=False,
                            )
                            for c in range(8):
                                nc.tensor.matmul(
                                    psh,
                                    xt[:, c, :],
                                    w_sb[:, m, c, off : off + 512],
                                    start=False,
                                    stop=(c == 7),
                                )
                        nc.scalar.activation(
                            dst, ps, func=mybir.ActivationFunctionType.Copy,
                        )
                        qkv_sb.append(dst)
                    q_sb, k_sb, v_sb = qkv_sb

                    # ---- scores[s,h,t] = sum_d q[s,h,d]*k[s,t,d] ----
                    prod = prodpool.tile([PT, H, H, DK], dt.bfloat16, tag="prod")
                    qb = bass.AP(
                        tensor=q_sb.tensor,
                        offset=q_sb.offset,
                        ap=[q_sb.ap[0], [DK, H], [0, H], [1, DK]],
                    )  # [p, 16h, 16t(bcast), 64d]
                    kb = bass.AP(
                        tensor=k_sb.tensor,
                        offset=k_sb.offset,
                        ap=[k_sb.ap[0], [0, H], [DK, H], [1, DK]],
                    )  # [p, 16h(bcast), 16t, 64d]
                    nc.vector.tensor_mul(prod, qb, kb)
                    w_ = DK
                    while w_ > 2:
                        w_ //= 2
                        nc.vector.tensor_add(
                            prod[:, :, :, 0:w_], prod[:, :, :, 0:w_],
                            prod[:, :, :, w_ : 2 * w_],
                        )
                    scores = smallpool.tile([PT, H, H], dt.float32, tag="scores")
                    p0 = bass.AP(
                        tensor=prod.tensor, offset=prod.offset,
                        ap=[prod.ap[0], [H * DK, H], [DK, H]],
                    )
                    p1 = bass.AP(
                        tensor=prod.tensor, offset=prod.offset + 1,
                        ap=[prod.ap[0], [H * DK, H], [DK, H]],
                    )
                    nc.vector.tensor_add(scores, p0, p1)

                    # ---- exp (softmax numerator; 1/sqrt(D) in the scale) ----
                    probs = smallpool.tile([PT, H, H], dt.bfloat16, tag="probs")
                    nc.scalar.activation(
                        probs, scores, func=mybir.ActivationFunctionType.Exp,
                        bias=zbias[:, 0:1], scale=float(inv_sqrt_d),
                    )
                    ctxA[it] = (v_sb, probs)

                # ---- stage2 for it-2: finish AV reduction, transpose,
                # O-projection, store ----
                if it >= 2:
                    j2 = it - 2
                    pv2b = ctxB.pop(j2)
                    nc.vector.tensor_add(
                        pv2b[:, :, :, 0:2], pv2b[:, :, :, 0:2],
                        pv2b[:, :, :, 2:4],
                    )
                    ao_bf = bigpool.tile([PT, D], dt.bfloat16, tag="aobf")
                    ao3 = ao_bf.rearrange("p (h d) -> p h d", h=H)
                    v0 = bass.AP(
                        tensor=pv2b.tensor, offset=pv2b.offset,
                        ap=[pv2b.ap[0], [DK * 8, H], [8, DK]],
                    )
                    v1 = bass.AP(
                        tensor=pv2b.tensor, offset=pv2b.offset + 1,
                        ap=[pv2b.ap[0], [DK * 8, H], [8, DK]],
                    )
                    nc.vector.tensor_add(ao3, v0, v1)
                    sub = j2 % SG
                    if sub == 0:
                        fout = bigpool.tile([PT, SG, D], dt.bfloat16, tag="fout")
                        # absorbs the WAR-vs-store wait into DVE's clock
                        nc.vector.memset(fout[:, 0, 0:1], 0.0)

                    aoT = bigpool.tile([PT, 8, PT], dt.bfloat16, tag="aoT")
                    tp = tpsum.tile([PT, D], dt.bfloat16, tag="tpsum")
                    for c in range(8):
                        nc.tensor.transpose(
                            tp[:, c * PT : (c + 1) * PT],
                            ao_bf[:, c * PT : (c + 1) * PT],
                            ident,
                        )
                    nc.scalar.activation(
                        aoT.rearrange("p c s -> p (c s)"),
                        tp, func=mybir.ActivationFunctionType.Copy,
                    )
                    ps = ppsum.tile([PT, D], dt.float32, tag="ppsum")
                    for half in range(2):
                        off = half * 512
                        psh = ps[:, off : off + 512]
                        nc.tensor.matmul(
                            psh,
                            ones_sb,
                            bias_sb[:, 3 * D + off : 3 * D + off + 512],
                            start=True,
                            stop=False,
                        )
                        for c in range(8):
                            nc.tensor.matmul(
                                psh,
                                aoT[:, c, :],
                                w_sb[:, 3, c, off : off + 512],
                                start=False,
                                stop=(c == 7),
                            )
                    nc.scalar.activation(
                        fout[:, sub, :], ps,
                        func=mybir.ActivationFunctionType.Copy,
                    )
                    if sub == SG - 1:
                        gt = j2 - sub  # first tile of the store group
                        nc.sync.dma_start(
                            out=out[:, gt * D : (gt + SG) * D],
                            in_=fout.rearrange("p j o -> p (j o)"),
                        )

    nc.compile()
    return nc


def _host_prep_w(Wq, bq, Wk, bk, Wv, bv, Wo, bo):
    perm = (np.arange(D).reshape(H, DK).T).reshape(-1)  # perm[d*16+t] = t*64+d
    wall = np.concatenate(
        [Wq.T, Wk.T, Wv.T[:, perm], Wo.T], axis=1
    ).astype(BF16)  # [D, 4D]
    # repack to SBUF layout: wall_sb[p, (m c o)] = wall[c*128+p, m*1024+o]
    wall_sb = np.ascontiguousarray(
        wall.reshape(8, PT, 4, D).transpose(1, 2, 0, 3).reshape(PT, 4 * 8 * D)
    )
    bias4 = np.concatenate([bq, bk, bv[perm], bo]).astype(BF16)[None, :]
    return wall_sb, np.ascontiguousarray(bias4)


def host_pack_x(x_core):
    """x_core: [TPC, D] fp32/bf16 for ONE core -> [PT, NT*8*PT] bf16 in the
    on-device xT layout: xT[p, ((g*LG+l)*8+c)*PT+s] = x[(g*LG+l)*PT+s, c*PT+p]
    i.e. grouped by LG-tile load group, channel-chunk major, token minor."""
    xg = x_core.reshape(NGRP, LG * PT, 8, PT)        # [g, s_local, c, p]
    xg = xg.transpose(3, 0, 2, 1)                    # [p, g, c, s_local]
    # within a group the SBUF tile is [p, l, c, s]: split s_local = (l, s)
    xg = xg.reshape(PT, NGRP, 8, LG, PT).transpose(0, 1, 3, 2, 4)
    return np.ascontiguousarray(xg.reshape(PT, NT * 8 * PT)).astype(BF16)


def host_unpack_out(res_core):
    """res_core: [PT, NT*D] bf16 -> [TPC, D] fp32."""
    o = res_core.reshape(PT, NT, D).transpose(1, 0, 2)
    return o.reshape(TPC, D).astype(np.float32)


def _make_runner(nc):
    """fast-dispatch jit-of-shard_map runner over 8 cores with
    device-created, donated output buffers."""
    install_neuronx_cc_hook()

    partition_name = (
        nc.partition_id_tensor.name if nc.partition_id_tensor else None
    )
    in_names, out_names, out_avals = [], [], []
    for alloc in nc.m.functions[0].allocations:
        if not isinstance(alloc, mybir.MemoryLocationSet):
            continue
        name = alloc.memorylocations[0].name
        if alloc.kind == "ExternalInput":
            if name != partition_name:
                in_names.append(name)
        elif alloc.kind == "ExternalOutput":
            out_names.append(name)
            shape = tuple(alloc.tensor_shape)
            dtype = mybir.dt.np(alloc.dtype)
            out_avals.append(jax.core.ShapedArray(shape, dtype))
    assert in_names == ["xT"] and out_names == ["out"], (in_names, out_names)

    all_in_names = in_names + out_names
    if partition_name is not None:
        all_in_names.append(partition_name)

    def _body(*args):
        operands = list(args)
        if partition_name is not None:
            operands.append(partition_id_tensor())
        outs = _bass_exec_p.bind(
            *operands,
            out_avals=tuple(out_avals),
            in_names=tuple(all_in_names),
            out_names=tuple(out_names),
            lowering_input_output_aliases=(),
            sim_require_finite=True,
            sim_require_nnan=True,
            nc=nc,
        )
        return tuple(outs)

    devices = jax.devices()[:NCORES]
    mesh = Mesh(np.asarray(devices), ("core",))
    sh = NamedSharding(mesh, PartitionSpec("core"))
    ishape = (NCORES * PT, NT * 8 * PT)
    oshape = (NCORES * PT, NT * D)
    odtype = out_avals[0].dtype

    def _compile():
        return (
            jax.jit(
                shard_map(
                    _body, mesh=mesh,
                    in_specs=(PartitionSpec("core"),) * 2,
                    out_specs=(PartitionSpec("core"),),
                    check_rep=False,
                ),
                donate_argnums=(1,),
                keep_unused=True,
            )
            .lower(
                jax.ShapeDtypeStruct(ishape, BF16, sharding=sh),
                jax.ShapeDtypeStruct(oshape, odtype, sharding=sh),
            )
            .compile()
        )

    try:
        sharded = fast_dispatch_compile(_compile)
    except Exception:
        sharded = jax.jit(
            shard_map(
                _body, mesh=mesh,
                in_specs=(PartitionSpec("core"),) * 2,
                out_specs=(PartitionSpec("core"),),
                check_rep=False,
            ),
            donate_argnums=(1,),
            keep_unused=True,
        )

    zeros_maker = jax.jit(
        lambda: jnp.zeros(oshape, odtype), out_shardings=sh,
    )

    obuf_cell = {"o": []}

    def run(x_chunks):
        """x_chunks: list of NCHUNK np arrays [NCORES*PT, NT*8*PT] bf16."""
        obufs = obuf_cell["o"]
        while len(obufs) < len(x_chunks):
            obufs.append(zeros_maker())
        outs = []
        for j, xc in enumerate(x_chunks):
            xd = jax.device_put(xc, sh)
            try:
                (o,) = sharded(xd, obufs[j])
            except Exception:
                (o,) = sharded(xd, zeros_maker())
            try:
                o.copy_to_host_async()
            except Exception:
                pass
            outs.append(o)
        res = [np.asarray(o) for o in outs]
        obuf_cell["o"] = outs  # donated next call (overwritten fully)
        return res

    return run


def kernel(x, Wq, bq, Wk, bk, Wv, bv, Wo, bo):
    x = np.asarray(x, dtype=np.float32)
    arrs = [np.asarray(a, dtype=np.float32)
            for a in (Wq, bq, Wk, bk, Wv, bv, Wo, bo)]

    wkey = hashlib.sha256(b"".join(a.tobytes() for a in arrs)).hexdigest()
    if _CACHE.get("wkey") != wkey:
        wall_sb, bias4 = _host_prep_w(*arrs)
        nc = _build_nc(wall_sb, bias4)
        _CACHE.update(
            wkey=wkey, nc=nc, run=_make_runner(nc),
        )

    x4 = x.reshape(NCORES, NCHUNK, TPC, D)
    x_chunks = [
        np.concatenate(
            [host_pack_x(x4[core, j]) for core in range(NCORES)], axis=0
        )
        for j in range(NCHUNK)
    ]
    res = _CACHE["run"](x_chunks)  # NCHUNK x [NCORES*PT, NT*D] bf16
    out = np.empty((NCORES, NCHUNK, TPC, D), np.float32)
    for j in range(NCHUNK):
        r = res[j].reshape(NCORES, PT, NT * D)
        for core in range(NCORES):
            out[core, j] = host_unpack_out(r[core])
    return out.reshape(B, S, D)
